# revision 1
# baseline (speedup 1.0000x reference)
"""Trainium2 Bass kernel for nn_BeansAttentionBlock (sparse attention block).

Strategy
--------
8 cores = 4 batches x 2 token-halves.  Each core:
  - gets its batch's x (feature-major, token-rolled so its query half is
    always local columns 0..TQ), computes LN1 + QKV for the full batch
    (K/V need all tokens), then dense masked attention for all 12 heads
    over its 514-column query block, then proj + LN2 + MLP for its half.
  - The routed kNN gather (with duplicate routes) is folded into a dense
    attention with a *multiplicity mask*:  E = exp(Q.K) * mult, where
    mult[k,q] = #occurrences of key k in query q's route list.  This is
    mathematically exact (softmax over 16 slots == mult-weighted dense
    softmax) and turns the gather into pure matmuls.
All matmuls run as float32r (full-rate fp32 PE mode, fp32 accumulation).
"""

import contextlib

import numpy as np

import concourse.bass as bass
import concourse.tile as tile
from concourse import bacc, mybir
from concourse.bass_utils import run_bass_kernel_spmd

F32 = mybir.dt.float32
F32R = mybir.dt.float32r
BF16 = mybir.dt.bfloat16
AF = mybir.ActivationFunctionType
ALU = mybir.AluOpType

# problem sizes (hardcoded per harness contract)
B, P, KN, D, H = 4, 1024, 16, 768, 12
HD = D // H          # 64
S = P + 1            # 1025
FT = D // 128        # 6 feature tiles
TOK = 1152           # padded key/token count = 9*128
NKT = TOK // 128     # 9 key tiles
TQ = 516             # per-core query block (2 x 258; fp32r needs even N)
QN = 258             # query chunk
DFF = 4 * D          # 3072
MT = DFF // 128      # 24
VW = H * (HD + 1)    # 780  (per-ktile width of V+ones layout)

TRACE = False        # test.py may set kernel.TRACE = True for profiling
LAST_EXEC_NS = None
LAST_RES = None

_STATE = {}


def _emit(nc, tc, ctx, t):
    """Emit the whole per-core program.  t = dict of dram tensor APs."""

    def pool(stack, name, bufs, space="SBUF"):
        return stack.enter_context(
            tc.tile_pool(name=name, bufs=bufs, space=space))

    def emit_ln(src, dst, T, chunks, stack):
        """LayerNorm stats via PE column-reductions; normalize via DVE."""
        sq_pool = pool(stack, "sq", 2)
        stat_ps = pool(stack, "stat_ps", 2, space="PSUM")
        bc_ps = pool(stack, "bc_ps", 2, space="PSUM")
        small = pool(stack, "small", 2)
        tmp_pool = pool(stack, "ln_tmp", 3)
        for (o, w) in chunks:
            mean_ps = stat_ps.tile([2, w], F32, tag="mean", name="mean_ps")
            sqm_ps = stat_ps.tile([2, w], F32, tag="sqm", name="sqm_ps")
            for ft in range(FT):
                sl = src[:, ft * T + o : ft * T + o + w]
                nc.tensor.matmul(mean_ps[:], (c768[:]), (sl),
                                 start=(ft == 0), stop=(ft == FT - 1))
            for ft in range(FT):
                sl = src[:, ft * T + o : ft * T + o + w]
                sq = sq_pool.tile([128, w], F32R, tag="sq", name="sq")
                nc.scalar.activation(sq[:], sl, AF.Square)
                nc.tensor.matmul(sqm_ps[:], (c768[:]), (sq[:]),
                                 start=(ft == 0), stop=(ft == FT - 1))
            mean_sb = small.tile([1, w], F32R, tag="mean_sb", name="mean_sb")
            nc.vector.tensor_copy(mean_sb[:], mean_ps[0:1, :])
            m2 = small.tile([1, w], F32, tag="m2", name="m2")
            nc.vector.tensor_mul(m2[:], mean_ps[0:1, :], mean_sb[:])
            var = small.tile([1, w], F32, tag="var", name="var")
            nc.vector.tensor_sub(var[:], sqm_ps[0:1, :], m2[:])
            std = small.tile([1, w], F32, tag="std", name="std")
            nc.scalar.activation(std[:], var[:], AF.Sqrt, bias=eps_sb[:])
            rstd_f = small.tile([1, w], F32, tag="rstd_f", name="rstd_f")
            nc.vector.reciprocal_approx_fast(rstd_f[:], std[:])
            rstd = small.tile([1, w], F32R, tag="rstd", name="rstd")
            nc.vector.tensor_copy(rstd[:], rstd_f[:])
            mb = bc_ps.tile([128, w], F32, tag="mb", name="mb")
            rb = bc_ps.tile([128, w], F32, tag="rb", name="rb")
            nc.tensor.matmul(mb[:], (ones_row[:, :]), (mean_sb[:]),
                             start=True, stop=True)
            nc.tensor.matmul(rb[:], (ones_row[:, :]), (rstd[:]),
                             start=True, stop=True)
            for ft in range(FT):
                sl = src[:, ft * T + o : ft * T + o + w]
                dl = dst[:, ft * T + o : ft * T + o + w]
                tmp = tmp_pool.tile([128, w], F32, tag="tmp", name="ln_tmp")
                nc.vector.tensor_sub(tmp[:], sl, mb[:])
                nc.vector.tensor_mul(dl, tmp[:], rb[:])

    # ================= S0: whole-kernel scope =========================
    pers = pool(ctx, "pers", 1)
    x2_sb = pers.tile([128, FT * TQ], F32R, tag="x2_sb", name="x2_sb")
    bq_sb = pers.tile([128, FT], F32, tag="bq_sb", name="bq_sb")
    bk_sb = pers.tile([128, FT], F32, tag="bk_sb", name="bk_sb")
    bvrow_sb = pers.tile([1, D], F32R, tag="bvrow_sb", name="bvrow_sb")
    bv_sb = pers.tile([128, D], F32, tag="bv_sb", name="bv_sb")
    pbrow_sb = pers.tile([1, D], F32R, tag="pbrow_sb", name="pbrow_sb")
    b1_sb = pers.tile([128, MT], F32, tag="b1_sb", name="b1_sb")
    b2row_sb = pers.tile([1, D], F32R, tag="b2row_sb", name="b2row_sb")
    ones_tq = pers.tile([1, TQ], F32R, tag="ones_tq", name="ones_tq")
    ones_row = pers.tile([1, 128], F32R, tag="ones_row", name="ones_row")
    c768 = pers.tile([128, 2], F32R, tag="c768", name="c768")
    eps_sb = pers.tile([1, 1], F32, tag="eps_sb", name="eps_sb")

    nc.gpsimd.memset(eps_sb[:], 1e-5)
    nc.sync.dma_start(ones_row[:], t["ones_r"][:, :])
    nc.sync.dma_start(c768[:], t["c768_r"][:, :])
    nc.sync.dma_start(bq_sb[:], t["bq"][:, :])
    nc.sync.dma_start(bk_sb[:], t["bk"][:, :])
    nc.sync.dma_start(bvrow_sb[:], t["bv"][:, :])
    nc.sync.dma_start(pbrow_sb[:], t["pbrow"][:, :])
    nc.sync.dma_start(b1_sb[:], t["b1"][:, :])
    nc.sync.dma_start(b2row_sb[:], t["b2row"][:, :])
    nc.sync.dma_start(ones_tq[:], t["ones_tq"][:, :])

    # ============ S1: QKV + attention + proj scope ====================
    with contextlib.ExitStack() as s1:
        p1 = pool(s1, "p1", 1)
        xq_sb = p1.tile([128, FT * TQ], F32, tag="xq_sb", name="xq_sb")
        q_sb = p1.tile([128, FT * TQ], BF16, tag="q_sb", name="q_sb")
        k_sb = p1.tile([128, FT * TOK], BF16, tag="k_sb", name="k_sb")
        v_sb = p1.tile([128, NKT * VW], BF16, tag="v_sb", name="v_sb")
        attn_sb = p1.tile([128, FT * TQ], BF16, tag="attn_sb", name="attn_sb")

        # -------- S2: LN1 + QKV ---------------------------------------
        with contextlib.ExitStack() as s2:
            p2 = pool(s2, "p2", 1)
            x_sb = p2.tile([128, FT * TOK], F32R, tag="x_sb", name="x_sb")
            xn_sb = p2.tile([128, FT * TOK], BF16, tag="xn_sb",
                            name="xn_sb")
            for ft in range(FT):
                nc.sync.dma_start(x_sb[:, ft * TOK : (ft + 1) * TOK],
                                  t["x_fm"][ft * 128 : (ft + 1) * 128, :])
            # residual slice of x (before LN1 overwrites x_sb in place)
            for ft in range(FT):
                nc.scalar.copy(xq_sb[:, ft * TQ : (ft + 1) * TQ],
                               x_sb[:, ft * TOK : ft * TOK + TQ])

            with contextlib.ExitStack() as sln:
                emit_ln(x_sb, xn_sb, TOK,
                        [(0, 384), (384, 384), (768, 384)], sln)

            wq_pool = pool(s2, "wqkv", FT)
            slabs = []
            for k in range(FT):
                sl = wq_pool.tile([128, 3 * D], BF16, tag="wslab",
                                  name=f"wslab{k}")
                nc.sync.dma_start(sl[:], t["qkv_w"][k * 128 : (k + 1) * 128, :])
                slabs.append(sl)

            qkv_ps = pool(s2, "qkv_ps", 2, space="PSUM")

            # bias_v broadcast to [128, D]
            bv_ps = qkv_ps.tile([128, 3, 512], F32, tag="ps", name="bv_ps")
            for ch in range(2):
                nc.tensor.matmul(bv_ps[:, ch, 0:384], (ones_row[:, :]),
                                 (bvrow_sb[:, ch * 384 : (ch + 1) * 384]),
                                 start=True, stop=True)
                nc.vector.tensor_copy(bv_sb[:, ch * 384 : (ch + 1) * 384],
                                      bv_ps[:, ch, 0:384])

            # Q: out [f, tq]  (2 chunks of 257)
            for m in range(FT):
                ps = qkv_ps.tile([128, 3, 512], F32, tag="ps", name="q_ps")
                for k in range(FT):
                    for qc in range(2):
                        nc.tensor.matmul(
                            ps[:, qc, 0:QN],
                            (slabs[k][:, m * 128 : (m + 1) * 128]),
                            (xn_sb[:, k * TOK + qc * QN :
                                      k * TOK + (qc + 1) * QN]),
                            start=(k == 0), stop=(k == FT - 1))
                nc.scalar.activation(
                    q_sb[:, m * TQ : (m + 1) * TQ].rearrange(
                        "p (a b) -> p a b", a=2),
                    ps[:, 0:2, 0:QN], AF.Identity, bias=bq_sb[:, m : m + 1])

            # K: out [f, tok]  (3 chunks of 384)
            for m in range(FT):
                ps = qkv_ps.tile([128, 3, 512], F32, tag="ps", name="k_ps")
                for k in range(FT):
                    for ch in range(3):
                        nc.tensor.matmul(
                            ps[:, ch, 0:384],
                            (slabs[k][:, D + m * 128 : D + (m + 1) * 128]),
                            (xn_sb[:, k * TOK + ch * 384 :
                                      k * TOK + (ch + 1) * 384]),
                            start=(k == 0), stop=(k == FT - 1))
                nc.vector.tensor_scalar_add(
                    k_sb[:, m * TOK : (m + 1) * TOK].rearrange(
                        "p (a b) -> p a b", a=3),
                    ps[:, :, 0:384], bk_sb[:, m : m + 1])

            # V: token-major, interleaved [v_h(64) | 1] per head.
            for tt in range(NKT):
                vv = v_sb[:, tt * VW : (tt + 1) * VW].rearrange(
                    "p (h s) -> p h s", h=H)
                nc.sync.dma_start(
                    vv[:, :, HD : HD + 1],
                    t["vones"][:, :].rearrange("p (h s) -> p h s", h=H))
                ps = qkv_ps.tile([128, 3, 512], F32, tag="ps", name="v_ps")
                for k in range(FT):
                    for ch in range(2):
                        nc.tensor.matmul(
                            ps[:, ch, 0:384],
                            (xn_sb[:, k * TOK + tt * 128 :
                                      k * TOK + (tt + 1) * 128]),
                            (slabs[k][:, 2 * D + ch * 384 :
                                        2 * D + (ch + 1) * 384]),
                            start=(k == 0), stop=(k == FT - 1))
                for ch in range(2):
                    out = v_sb[:, tt * VW + ch * 6 * (HD + 1) :
                               tt * VW + (ch + 1) * 6 * (HD + 1)]
                    out = out.rearrange("p (h s) -> p h s", h=6)[:, :, 0:HD]
                    nc.vector.tensor_add(
                        out,
                        ps[:, ch, 0:384].rearrange("p (h s) -> p h s", h=6),
                        bv_sb[:, ch * 384 : (ch + 1) * 384].rearrange(
                            "p (h s) -> p h s", h=6))

        # -------- S3: attention ---------------------------------------
        with contextlib.ExitStack() as s3:
            p3 = pool(s3, "p3", 1)
            mult_sb = p3.tile([128, NKT, 2, QN], BF16, tag="mult_sb",
                              name="mult_sb")
            for kt in range(NKT):
                nc.sync.dma_start(
                    mult_sb[:, kt, :, :],
                    t["multT"][kt, :, :].rearrange("p (a b) -> p a b", a=2))

            scp = pool(s3, "scp", 2, space="PSUM")
            avp = pool(s3, "avp", 2, space="PSUM")
            e_pool = pool(s3, "e", 6)
            e2_pool = pool(s3, "e2", 6)
            small3 = pool(s3, "small3", 4)
            stage_pool = pool(s3, "stage", 4)

            for hp in range(H // 2):
                ft = hp
                avs = [avp.tile([65, 2, 512], F32, tag="av",
                                name=f"av{hp}_{i}") for i in range(2)]
                for kt in range(NKT):
                    scs = [scp.tile([128, 2, 512], F32, tag="sc",
                                    name=f"sc{kt}_{i}") for i in range(2)]
                    for qc in range(2):
                        for sub in range(2):
                            row = sub * HD
                            nc.tensor.matmul(
                                scs[sub][:, qc, 0:QN],
                                (k_sb[row : row + HD, ft * TOK + kt * 128 :
                                        ft * TOK + (kt + 1) * 128]),
                                (q_sb[row : row + HD, ft * TQ + qc * QN :
                                        ft * TQ + (qc + 1) * QN]),
                                start=True, stop=True,
                                tile_position=(row, 0))
                    e2s = []
                    for sub in range(2):
                        e = e_pool.tile([128, 2, QN], BF16, tag="e", name="e")
                        nc.scalar.activation(e[:], scs[sub][:, :, 0:QN],
                                             AF.Exp)
                        e2 = e2_pool.tile([128, 2, QN], BF16, tag="e2",
                                          name="e2")
                        eng = (nc.gpsimd if (kt * 2 + sub) % 4 == 3
                               else nc.vector)
                        eng.tensor_mul(e2[:], e[:], mult_sb[:, kt, :, :])
                        e2s.append(e2)
                    for qc in range(2):
                        for sub in range(2):
                            h = 2 * hp + sub
                            nc.tensor.matmul(
                                avs[sub][:, qc, 0:QN],
                                (v_sb[:, kt * VW + h * (HD + 1) :
                                        kt * VW + (h + 1) * (HD + 1)]),
                                (e2s[sub][:, qc, :]),
                                start=(kt == 0), stop=(kt == NKT - 1))
                for sub in range(2):
                    h = 2 * hp + sub
                    row = sub * HD
                    for qc in range(2):
                        den_sb = small3.tile([1, QN], F32R, tag="den",
                                             name="den_sb")
                        nc.vector.tensor_copy(den_sb[:],
                                              avs[sub][HD : HD + 1, qc, 0:QN])
                        rb = scp.tile([64, QN], F32, tag="sc", name="rb")
                        nc.tensor.matmul(rb[:], (ones_row[:, 0:HD]),
                                         (den_sb[:]), start=True, stop=True)
                        rb_sb = stage_pool.tile([64, QN], F32, tag="rb_sb",
                                                name="rb_sb")
                        nc.vector.reciprocal_approx_fast(rb_sb[:], rb[:])
                        dst = attn_sb[row : row + HD, ft * TQ + qc * QN :
                                      ft * TQ + (qc + 1) * QN]
                        if sub == 0:
                            nc.vector.tensor_mul(dst, avs[sub][0:HD, qc, 0:QN],
                                                 rb_sb[:])
                        else:
                            st = stage_pool.tile([HD, QN], BF16, tag="stage",
                                                 name="stage")
                            nc.vector.tensor_mul(st[:],
                                                 avs[sub][0:HD, qc, 0:QN],
                                                 rb_sb[:])
                            nc.sync.dma_start(dst, st[:])

        # -------- S4: proj + residual ---------------------------------
        with contextlib.ExitStack() as s4:
            pw_pool = pool(s4, "pw", FT)
            pslabs = []
            for k in range(FT):
                sl = pw_pool.tile([128, D], BF16, tag="pwslab",
                                  name=f"pwslab{k}")
                nc.sync.dma_start(sl[:],
                                  t["proj_w"][k * 128 : (k + 1) * 128, :])
                pslabs.append(sl)
            pr_ps = pool(s4, "pr_ps", 2, space="PSUM")
            for m in range(FT):
                ps = pr_ps.tile([128, 2, 512], F32, tag="pr", name="pr_ps")
                for qc in range(2):
                    nc.tensor.matmul(
                        ps[:, qc, 0:QN],
                        (pbrow_sb[:, m * 128 : (m + 1) * 128]),
                        (ones_tq[:, qc * QN : (qc + 1) * QN]),
                        start=True, stop=False)
                for k in range(FT):
                    for qc in range(2):
                        nc.tensor.matmul(
                            ps[:, qc, 0:QN],
                            (pslabs[k][:, m * 128 : (m + 1) * 128]),
                            (attn_sb[:, k * TQ + qc * QN :
                                       k * TQ + (qc + 1) * QN]),
                            start=False, stop=(k == FT - 1))
                nc.vector.tensor_add(
                    x2_sb[:, m * TQ : (m + 1) * TQ].rearrange(
                        "p (a b) -> p a b", a=2),
                    ps[:, :, 0:QN],
                    xq_sb[:, m * TQ : (m + 1) * TQ].rearrange(
                        "p (a b) -> p a b", a=2))

    # ============ S5: LN2 + MLP =======================================
    with contextlib.ExitStack() as s5:
        p5 = pool(s5, "p5", 1)
        h1_sb = p5.tile([128, MT * TQ], BF16, tag="h1_sb", name="h1_sb")

        with contextlib.ExitStack() as s5a:
            p5a = pool(s5a, "p5a", 1)
            xn2_sb = p5a.tile([128, FT * TQ], BF16, tag="xn2_sb",
                              name="xn2_sb")
            with contextlib.ExitStack() as sln2:
                emit_ln(x2_sb, xn2_sb, TQ, [(0, QN), (QN, QN)], sln2)

            w1_pool = pool(s5a, "w1", FT)
            w1slabs = []
            for k in range(FT):
                sl = w1_pool.tile([128, DFF], BF16, tag="w1slab",
                                  name=f"w1slab{k}")
                nc.sync.dma_start(sl[:], t["w1"][k * 128 : (k + 1) * 128, :])
                w1slabs.append(sl)
            fc1_ps = pool(s5a, "fc1_ps", 3, space="PSUM")
            for m in range(MT):
                ps = fc1_ps.tile([128, 2, 512], F32, tag="fc1", name="fc1_ps")
                for k in range(FT):
                    for qc in range(2):
                        nc.tensor.matmul(
                            ps[:, qc, 0:QN],
                            (w1slabs[k][:, m * 128 : (m + 1) * 128]),
                            (xn2_sb[:, k * TQ + qc * QN :
                                      k * TQ + (qc + 1) * QN]),
                            start=(k == 0), stop=(k == FT - 1))
                nc.scalar.activation(
                    h1_sb[:, m * TQ : (m + 1) * TQ].rearrange(
                        "p (a b) -> p a b", a=2),
                    ps[:, 0:2, 0:QN], AF.Gelu, bias=b1_sb[:, m : m + 1])

        with contextlib.ExitStack() as s5b:
            p5b = pool(s5b, "p5b", 1)
            y_sb = p5b.tile([128, FT * TQ], F32, tag="y_sb", name="y_sb")
            w2_pool = pool(s5b, "w2", MT)
            w2slabs = []
            for k in range(MT):
                sl = w2_pool.tile([128, D], BF16, tag="w2slab",
                                  name=f"w2slab{k}")
                nc.sync.dma_start(sl[:], t["w2"][k * 128 : (k + 1) * 128, :])
                w2slabs.append(sl)
            fc2_ps = pool(s5b, "fc2_ps", 2, space="PSUM")
            for m in range(FT):
                ps = fc2_ps.tile([128, 2, 512], F32, tag="fc2", name="fc2_ps")
                for qc in range(2):
                    nc.tensor.matmul(
                        ps[:, qc, 0:QN],
                        (b2row_sb[:, m * 128 : (m + 1) * 128]),
                        (ones_tq[:, qc * QN : (qc + 1) * QN]),
                        start=True, stop=False)
                for k in range(MT):
                    for qc in range(2):
                        nc.tensor.matmul(
                            ps[:, qc, 0:QN],
                            (w2slabs[k][:, m * 128 : (m + 1) * 128]),
                            (h1_sb[:, k * TQ + qc * QN :
                                     k * TQ + (qc + 1) * QN]),
                            start=False, stop=(k == MT - 1))
                nc.vector.tensor_add(
                    y_sb[:, m * TQ : (m + 1) * TQ].rearrange(
                        "p (a b) -> p a b", a=2),
                    ps[:, :, 0:QN],
                    x2_sb[:, m * TQ : (m + 1) * TQ].rearrange(
                        "p (a b) -> p a b", a=2))

            for ft in range(FT):
                nc.sync.dma_start(t["out_fm"][ft * 128 : (ft + 1) * 128, :],
                                  y_sb[:, ft * TQ : (ft + 1) * TQ])


def _build():
    if "nc" in _STATE:
        return _STATE["nc"]
    nc = bacc.Bacc("TRN2", target_bir_lowering=False, debug=False,
                   num_devices=8)
    t = {
        "x_fm": nc.dram_tensor("x_fm", [D, TOK], F32R, kind="ExternalInput"),
        "ones_r": nc.dram_tensor("ones_r", [1, 128], F32R,
                                 kind="ExternalInput"),
        "c768_r": nc.dram_tensor("c768_r", [128, 2], F32R,
                                 kind="ExternalInput"),
        "vones": nc.dram_tensor("vones", [128, H], BF16,
                                kind="ExternalInput"),
        "multT": nc.dram_tensor("multT", [NKT, 128, TQ], BF16,
                                kind="ExternalInput"),
        "qkv_w": nc.dram_tensor("qkv_w", [D, 3 * D], BF16,
                                kind="ExternalInput"),
        "bq": nc.dram_tensor("bq", [128, FT], F32, kind="ExternalInput"),
        "bk": nc.dram_tensor("bk", [128, FT], F32, kind="ExternalInput"),
        "bv": nc.dram_tensor("bv", [1, D], F32R, kind="ExternalInput"),
        "proj_w": nc.dram_tensor("proj_w", [D, D], BF16, kind="ExternalInput"),
        "pbrow": nc.dram_tensor("pbrow", [1, D], F32R, kind="ExternalInput"),
        "w1": nc.dram_tensor("w1", [D, DFF], BF16, kind="ExternalInput"),
        "b1": nc.dram_tensor("b1", [128, MT], F32, kind="ExternalInput"),
        "w2": nc.dram_tensor("w2", [DFF, D], BF16, kind="ExternalInput"),
        "b2row": nc.dram_tensor("b2row", [1, D], F32R, kind="ExternalInput"),
        "ones_tq": nc.dram_tensor("ones_tq", [1, TQ], F32R,
                                  kind="ExternalInput"),
        "out_fm": nc.dram_tensor("out_fm", [D, TQ], F32,
                                 kind="ExternalOutput"),
    }
    t = {k: (v.ap() if hasattr(v, "ap") else v) for k, v in t.items()}
    with contextlib.ExitStack() as ctx:
        ctx.enter_context(nc.allow_low_precision(
            reason="float32r rounding of matmul operands is intentional"))
        tc = ctx.enter_context(tile.TileContext(nc))
        _emit(nc, tc, ctx, t)
    nc.compile()
    _STATE["nc"] = nc
    return nc


def _pp(a, dt=np.float32):
    return np.ascontiguousarray(np.asarray(a, dtype=dt))


def _host_prep(x, routes, qkv_w, qkv_b, proj_w, proj_b, ln1_g, ln1_b,
               ln2_g, ln2_b, mlp_w1, mlp_b1, mlp_w2, mlp_b2):
    x = _pp(x)
    routes = np.asarray(routes).astype(np.int64)
    qkv_w, qkv_b = _pp(qkv_w), _pp(qkv_b)
    proj_w, proj_b = _pp(proj_w), _pp(proj_b)
    ln1_g, ln1_b, ln2_g, ln2_b = map(_pp, (ln1_g, ln1_b, ln2_g, ln2_b))
    mlp_w1, mlp_b1, mlp_w2, mlp_b2 = map(_pp, (mlp_w1, mlp_b1, mlp_w2, mlp_b2))

    scale = HD ** -0.5
    w_eff = (qkv_w * ln1_g[:, None]).astype(np.float32)
    b_eff = (ln1_b @ qkv_w + qkv_b).astype(np.float32)
    w_eff[:, :D] *= scale
    b_eff[:D] *= scale
    w1_eff = (mlp_w1 * ln2_g[:, None]).astype(np.float32)
    b1_eff = (ln2_b @ mlp_w1 + mlp_b1).astype(np.float32)

    # multiplicity mask  M[k_global, q_global]
    M = np.zeros((S, S), np.float32)
    M[:, 0] = 1.0
    np.add.at(M, ((routes + 1).ravel(),
                  np.repeat(np.arange(1, S), KN)), 1.0)

    def col(v, nt):   # [nt*128] -> [128, nt] per-partition bias layout
        return _pp(v.reshape(nt, 128).T)

    import ml_dtypes
    bf16 = ml_dtypes.bfloat16
    shared = {
        "ones_r": np.ones((1, 128), np.float32),
        "c768_r": np.full((128, 2), 1.0 / D, np.float32),
        "vones": np.ones((128, H), bf16),
        "qkv_w": np.ascontiguousarray(w_eff.astype(bf16)),
        "bq": col(b_eff[:D], FT),
        "bk": col(b_eff[D:2 * D], FT),
        "bv": _pp(b_eff[2 * D:].reshape(1, D)),
        "proj_w": np.ascontiguousarray(proj_w.astype(bf16)),
        "pbrow": _pp(proj_b.reshape(1, D)),
        "w1": np.ascontiguousarray(w1_eff.astype(bf16)),
        "b1": col(b1_eff, MT),
        "w2": np.ascontiguousarray(mlp_w2.astype(bf16)),
        "b2row": _pp(mlp_b2.reshape(1, D)),
        "ones_tq": np.ones((1, TQ), np.float32),
    }

    in_maps = []
    for c in range(8):
        b, half = c // 2, c % 2
        if half == 0:
            g = np.arange(S)
        else:
            g = np.concatenate([np.arange(513, S), np.arange(0, 513)])
        x_fm = np.zeros((D, TOK), np.float32)
        x_fm[:, :S] = x[b][g].T
        multT = np.zeros((TOK, TQ), np.float32)
        nreal = 513 if half == 0 else 512
        # local query j -> global token (513*half + j); local key i -> g[i]
        multT[:S, :nreal] = M[g][:, 513 * half : 513 * half + nreal]
        m = dict(shared)
        m["x_fm"] = x_fm
        m["multT"] = np.ascontiguousarray(multT.reshape(NKT, 128, TQ).astype(bf16))
        in_maps.append(m)
    return in_maps


def kernel(**inputs):
    global LAST_EXEC_NS
    nc = _build()
    in_maps = _host_prep(**inputs)
    res = run_bass_kernel_spmd(nc, in_maps, list(range(8)), trace=TRACE)
    LAST_EXEC_NS = res.exec_time_ns
    globals()["LAST_RES"] = res
    out = np.zeros((B, S, D), np.float32)
    for c in range(8):
        b, half = c // 2, c % 2
        y = res.results[c]["out_fm"]            # [768, 514]
        nreal = 513 if half == 0 else 512
        out[b, 513 * half : 513 * half + nreal, :] = y[:, :nreal].T
    return out

